# revision 5
# baseline (speedup 1.0000x reference)
"""Trainium2 Bass kernel: 2x2 depthwise blur + 2x downsample (stride 2, SAME).

Full input x (32, 512, 512, 3) f32 NHWC, kernel (2, 2) f32.
Output (32, 256, 256, 3) f32:
    out[b, i, j, c] = sum_{di, dj in {0,1}} kernel[di, dj] * x[b, 2i+di, 2j+dj, c]
(H=W=512 even, k=2, stride=2 -> SAME padding adds no rows/cols.)

Sharding: pure data parallelism, batch split 4 samples per core across 8
NeuronCores.  Per-core kernel is memory-bound (12.6MB in, 3.1MB out).
Raw Bass (no TileContext: its kernel-tail Drain emits more sync waits than
this walrus build can encode on a TPB_CTRL op).

Per 128-output-row tile: one contiguous 1.5MB DMA load on the SP HWDGE ring
(input rows 2i, 2i+1 are DRAM-adjacent and land on one partition), 4 DVE ops
combine the taps via strided SBUF access patterns (kernel weights baked as
immediates at trace time), one contiguous 384KB store on the ACT HWDGE ring.
Every tile gets its own SBUF buffers (120KB/partition total), so the only
synchronization is load-done -> compute and compute-done -> store.
"""

import sys
from contextlib import ExitStack

import numpy as np

if "/opt/trn_rl_repo" not in sys.path:
    sys.path.insert(0, "/opt/trn_rl_repo")

import concourse.bass as bass
import concourse.mybir as mybir
from concourse.bass_utils import run_bass_kernel_spmd

N_CORES = 8
B_FULL = 32
B_LOCAL = B_FULL // N_CORES  # 4
H, W, C = 512, 512, 3
HO, WO = H // 2, W // 2  # 256, 256
P = 128
ROW = W * C  # 1536 floats per input row
OROW = WO * C  # 768 floats per output row
N_TILES = B_LOCAL * HO // P  # 8 tiles of 128 output rows each

FP32 = mybir.dt.float32


def build_nc(k00: float, k01: float, k10: float, k11: float) -> bass.Bass:
    nc = bass.Bass()
    x = nc.dram_tensor("x", [B_LOCAL, H, W, C], FP32, kind="ExternalInput")
    y = nc.dram_tensor("y", [B_LOCAL, HO, WO, C], FP32, kind="ExternalOutput")

    # h = t*256 + p*2 + two : partition p holds input rows 2i, 2i+1 for
    # output row i = t*128 + p.  Each [128, 3072] slice is contiguous in DRAM.
    xv = x.rearrange("b (t p two) w c -> (b t) p (two w c)", p=P, two=2)
    yv = y.rearrange("b (t p) w c -> (b t) p (w c)", p=P)

    mult = mybir.AluOpType.mult
    add = mybir.AluOpType.add

    with ExitStack() as ctx:
        tin = [
            ctx.enter_context(nc.sbuf_tensor(f"tin{i}", [P, 2 * ROW], FP32))
            for i in range(N_TILES)
        ]
        acc = [
            ctx.enter_context(nc.sbuf_tensor(f"acc{i}", [P, OROW], FP32))
            for i in range(N_TILES)
        ]
        ld_sem = ctx.enter_context(nc.semaphore("ld_sem"))
        cp_sem = ctx.enter_context(nc.semaphore("cp_sem"))
        st_sem = ctx.enter_context(nc.semaphore("st_sem"))
        block = ctx.enter_context(nc.Block())

        @block.sync
        def _(sync):
            for i in range(N_TILES):
                sync.dma_start(out=tin[i][:], in_=xv[i]).then_inc(ld_sem, 16)

        @block.vector
        def _(vector):
            for i in range(N_TILES):
                vector.wait_ge(ld_sem, 16 * (i + 1))
                # [128, (r=row-of-pair, j=out col, q=col parity, c)]
                tv = tin[i].rearrange("p (r j q c) -> p r j q c", r=2, j=WO, q=2)
                av = acc[i].rearrange("p (j c) -> p j c", c=C)
                vector.tensor_scalar_mul(av, tv[:, 0, :, 0, :], k00)
                vector.scalar_tensor_tensor(
                    av, tv[:, 0, :, 1, :], k01, av, op0=mult, op1=add
                )
                vector.scalar_tensor_tensor(
                    av, tv[:, 1, :, 0, :], k10, av, op0=mult, op1=add
                )
                vector.scalar_tensor_tensor(
                    av, tv[:, 1, :, 1, :], k11, av, op0=mult, op1=add
                ).then_inc(cp_sem, 1)

        @block.scalar
        def _(scalar):
            for i in range(N_TILES):
                scalar.wait_ge(cp_sem, i + 1)
                scalar.dma_start(out=yv[i], in_=acc[i][:]).then_inc(st_sem, 16)

        # The NEFF can be executed more than once per load (NTFF profiling
        # replays it); semaphore values persist across executions, so the
        # sole final-completion waiter must also reset them for idempotence.
        @block.gpsimd
        def _(gpsimd):
            gpsimd.wait_ge(st_sem, 16 * N_TILES)
            nums = sorted(s.num for s in (ld_sem, cp_sem, st_sem))
            assert nums[-1] - nums[0] == 2, nums
            rng = range(nums[0], nums[-1] + 1)
            gpsimd.dma_reset(rng)
            gpsimd.sem_clear(rng)

    return nc


def run(x: np.ndarray, kernel: np.ndarray, trace: bool = False):
    """Shard, compile+run on 8 cores, gather.  Returns (output, BassKernelResults)."""
    x = np.ascontiguousarray(x, dtype=np.float32)
    kernel = np.asarray(kernel, dtype=np.float32)
    nc = build_nc(
        float(kernel[0, 0]), float(kernel[0, 1]),
        float(kernel[1, 0]), float(kernel[1, 1]),
    )
    core_ids = list(range(N_CORES))
    in_maps = [{"x": x[c * B_LOCAL : (c + 1) * B_LOCAL]} for c in core_ids]
    res = run_bass_kernel_spmd(nc, in_maps, core_ids, trace=trace)
    out = np.concatenate([res.results[c]["y"] for c in core_ids], axis=0)
    return out, res


def kernel(x: np.ndarray, kernel: np.ndarray) -> np.ndarray:
    out, _ = run(x, kernel, trace=False)
    return out


if __name__ == "__main__":
    xs = np.random.randn(B_FULL, H, W, C).astype(np.float32)
    ks = np.full((2, 2), 0.25, np.float32)
    out, _ = run(xs, ks)
    print(out.shape, out.dtype)


# revision 11
# speedup vs baseline: 1.1924x; 1.1924x over previous
"""Trainium2 Bass kernel: 2x2 depthwise blur + 2x downsample (stride 2, SAME).

Full input x (32, 512, 512, 3) f32 NHWC, kernel (2, 2) f32.
Output (32, 256, 256, 3) f32:
    out[b, i, j, c] = sum_{di, dj in {0,1}} kernel[di, dj] * x[b, 2i+di, 2j+dj, c]
(H=W=512 even, k=2, stride=2 -> SAME padding adds no rows/cols.)

Sharding: pure data parallelism, batch split 4 samples per core across 8
NeuronCores.  Per-core kernel is memory-bound (12.6MB in, 3.1MB out; measured
steady-state DMA ~415-430 GB/s vs the 435 GB/s SBUF-port ceiling).
Raw Bass (no TileContext: its kernel-tail Drain emits more sync waits than
this walrus build can encode on a TPB_CTRL op).

Per 128-output-row tile: two 768KB DMA loads on the SP HWDGE ring (even/odd
input rows; each partition's row is 6KB contiguous in DRAM), 4 DVE ops
combine the taps via strided SBUF access patterns (kernel weights baked as
immediates at trace time), one contiguous 384KB store on the ACT HWDGE ring.
The last tile's odd-row load/compute/store is additionally split into column
halves to shorten the serial (last load -> DVE -> store) kernel tail.
Every tile has its own SBUF buffers (120KB/partition total), so the only
synchronization is load-done -> compute and compute-done -> store.

Sync protocol notes (hard-won):
- A DMA's then_inc(sem, 16) is 16 independent per-SDMA-engine increments and
  engines skew by multiple DMAs, so mid-stream cumulative thresholds on a
  shared sem are racy; every load DMA gets its OWN sem (wait >= 16 is exact).
- The NEFF can be executed repeatedly (e.g. NTFF profiling replays it), so
  the kernel tail resets all sems (gpsimd dma_reset + sem_clear) after the
  final store completes, making the program idempotent.
"""

import sys
from contextlib import ExitStack

import numpy as np

if "/opt/trn_rl_repo" not in sys.path:
    sys.path.insert(0, "/opt/trn_rl_repo")

import concourse.bass as bass
import concourse.mybir as mybir
from concourse.bass_utils import run_bass_kernel_spmd

N_CORES = 8
B_FULL = 32
B_LOCAL = B_FULL // N_CORES  # 4
H, W, C = 512, 512, 3
HO, WO = H // 2, W // 2  # 256, 256
P = 128
ROW = W * C  # 1536 floats per input row
OROW = WO * C  # 768 floats per output row
N_TILES = B_LOCAL * HO // P  # 8 tiles of 128 output rows each

FP32 = mybir.dt.float32


def build_nc(k00: float, k01: float, k10: float, k11: float) -> bass.Bass:
    nc = bass.Bass()
    x = nc.dram_tensor("x", [B_LOCAL, H, W, C], FP32, kind="ExternalInput")
    y = nc.dram_tensor("y", [B_LOCAL, HO, WO, C], FP32, kind="ExternalOutput")

    # h = t*256 + p*2 + two : partition p holds input rows 2i (two=0) and
    # 2i+1 (two=1) for output row i = t*128 + p.  Each [128, two, 1536]
    # slice has 6KB contiguous per partition; the two=0/1 halves of a tile
    # are loaded as separate DMAs so DVE can start on the even rows while
    # the odd rows are still in flight (shortens the post-last-load chain).
    xv = x.rearrange("b (t p two) w c -> (b t) p two (w c)", p=P, two=2)
    yv = y.rearrange("b (t p) w c -> (b t) p (w c)", p=P)

    mult = mybir.AluOpType.mult
    add = mybir.AluOpType.add

    with ExitStack() as ctx:
        tin = [
            ctx.enter_context(nc.sbuf_tensor(f"tin{i}", [P, 2 * ROW], FP32))
            for i in range(N_TILES)
        ]
        acc = [
            ctx.enter_context(nc.sbuf_tensor(f"acc{i}", [P, OROW], FP32))
            for i in range(N_TILES)
        ]
        # One semaphore per load DMA: a DMA's 16 sem increments come from 16
        # independent SDMA engines with unbounded skew, so a cumulative
        # threshold on a shared sem can be reached while a slow engine still
        # has older DMAs in flight.  Per-DMA sems (wait >= 16) are exact.
        ld_sems = [
            ctx.enter_context(nc.semaphore(f"ld{i}_{r}"))
            for i in range(N_TILES)
            for r in range(2)
        ]
        cp_sem = ctx.enter_context(nc.semaphore("cp_sem"))
        st_sem = ctx.enter_context(nc.semaphore("st_sem"))
        block = ctx.enter_context(nc.Block(no_gpsimd_drain=True))

        # Extra sem for splitting the last tile's odd-row load in two: the
        # kernel tail is (last load -> DVE -> store) serial, so the final
        # quanta are halved to shorten it.
        ldx_sem = ctx.enter_context(nc.semaphore("ldx_sem"))
        LAST = N_TILES - 1
        HW_, HF = WO // 2, ROW // 2  # half of j range, half of a row in floats

        @block.sync
        def _(sync):
            for i in range(N_TILES):
                sync.dma_start(
                    out=tin[i][:, 0:ROW], in_=xv[i, :, 0]
                ).then_inc(ld_sems[2 * i], 16)
                if i < LAST:
                    sync.dma_start(
                        out=tin[i][:, ROW : 2 * ROW], in_=xv[i, :, 1]
                    ).then_inc(ld_sems[2 * i + 1], 16)
                else:
                    xr = xv[i, :, 1]
                    sync.dma_start(
                        out=tin[i][:, ROW : ROW + HF], in_=xr[:, 0:HF]
                    ).then_inc(ld_sems[2 * i + 1], 16)
                    sync.dma_start(
                        out=tin[i][:, ROW + HF : 2 * ROW], in_=xr[:, HF:ROW]
                    ).then_inc(ldx_sem, 16)

        @block.vector
        def _(vector):
            for i in range(N_TILES):
                # [128, (r=row-of-pair, j=out col, q=col parity, c)]
                tv = tin[i].rearrange("p (r j q c) -> p r j q c", r=2, j=WO, q=2)
                av = acc[i].rearrange("p (j c) -> p j c", c=C)
                vector.wait_ge(ld_sems[2 * i], 16)
                vector.tensor_scalar_mul(av, tv[:, 0, :, 0, :], k00)
                vector.scalar_tensor_tensor(
                    av, tv[:, 0, :, 1, :], k01, av, op0=mult, op1=add
                )
                vector.wait_ge(ld_sems[2 * i + 1], 16)
                if i < LAST:
                    vector.scalar_tensor_tensor(
                        av, tv[:, 1, :, 0, :], k10, av, op0=mult, op1=add
                    )
                    vector.scalar_tensor_tensor(
                        av, tv[:, 1, :, 1, :], k11, av, op0=mult, op1=add
                    ).then_inc(cp_sem, 1)
                else:
                    for lo, hi, sem in ((0, HW_, None), (HW_, WO, ldx_sem)):
                        if sem is not None:
                            vector.wait_ge(sem, 16)
                        vector.scalar_tensor_tensor(
                            av[:, lo:hi], tv[:, 1, lo:hi, 0, :], k10,
                            av[:, lo:hi], op0=mult, op1=add,
                        )
                        vector.scalar_tensor_tensor(
                            av[:, lo:hi], tv[:, 1, lo:hi, 1, :], k11,
                            av[:, lo:hi], op0=mult, op1=add,
                        ).then_inc(cp_sem, 1)

        @block.scalar
        def _(scalar):
            for i in range(LAST):
                scalar.wait_ge(cp_sem, i + 1)
                scalar.dma_start(out=yv[i], in_=acc[i][:]).then_inc(st_sem, 16)
            scalar.wait_ge(cp_sem, LAST + 1)
            scalar.dma_start(
                out=yv[LAST][:, 0 : C * HW_], in_=acc[LAST][:, 0 : C * HW_]
            ).then_inc(st_sem, 16)
            scalar.wait_ge(cp_sem, LAST + 2)
            scalar.dma_start(
                out=yv[LAST][:, C * HW_ : OROW], in_=acc[LAST][:, C * HW_ : OROW]
            ).then_inc(st_sem, 16)

        # The NEFF can be executed more than once per load (NTFF profiling
        # replays it); semaphore values persist across executions, so the
        # sole final-completion waiter must also reset them for idempotence.
        @block.gpsimd
        def _(gpsimd):
            # st_sem is cumulative but only its final total is consumed,
            # which is exact (all increments observed).
            gpsimd.wait_ge(st_sem, 16 * (N_TILES + 1))
            nums = sorted(s.num for s in (*ld_sems, cp_sem, st_sem, ldx_sem))
            assert nums[-1] - nums[0] == len(nums) - 1, nums
            rng = range(nums[0], nums[-1] + 1)
            gpsimd.dma_reset(rng)
            gpsimd.sem_clear(rng)

    return nc


def run(x: np.ndarray, kernel: np.ndarray, trace: bool = False):
    """Shard, compile+run on 8 cores, gather.  Returns (output, BassKernelResults)."""
    x = np.ascontiguousarray(x, dtype=np.float32)
    kernel = np.asarray(kernel, dtype=np.float32)
    nc = build_nc(
        float(kernel[0, 0]), float(kernel[0, 1]),
        float(kernel[1, 0]), float(kernel[1, 1]),
    )
    core_ids = list(range(N_CORES))
    in_maps = [{"x": x[c * B_LOCAL : (c + 1) * B_LOCAL]} for c in core_ids]
    res = run_bass_kernel_spmd(nc, in_maps, core_ids, trace=trace)
    out = np.concatenate([res.results[c]["y"] for c in core_ids], axis=0)
    return out, res


def kernel(x: np.ndarray, kernel: np.ndarray) -> np.ndarray:
    out, _ = run(x, kernel, trace=False)
    return out


if __name__ == "__main__":
    xs = np.random.randn(B_FULL, H, W, C).astype(np.float32)
    ks = np.full((2, 2), 0.25, np.float32)
    out, _ = run(xs, ks)
    print(out.shape, out.dtype)
